# revision 1
# baseline (speedup 1.0000x reference)
"""LDS forward kernel for Trainium2 (8 NeuronCores, data-parallel over batch).

Math: the reference LDS
    h_t = A*h_{t-1} + x_t @ B;  y_t = h_t @ C + sum_i M[:,0,i] x_{t-1-i}
with diagonal A and d_in == 1 is an exact causal convolution plus a
batch-independent bias:
    out[b,t,o] = sum_{d=0}^{t} Ktot[d,o] * x[b,t-d] + bias[t,o]
    Ktot[d,o]  = sum_s B[s] A[s]^d C[s,o]  (+ M[o,0,d-1] for d in 1..KX)
    bias[t,o]  = sum_s h0[s] A[s]^{t+1} C[s,o]
Ktot/bias are precomputed on host in float64 (cheap: T*S*O flops).

Device kernel per core (32 batch rows): blocked lower-triangular Toeplitz
matmul. The lag axis is blocked into 4 chunks of 128 (the PE contraction
dim). The moving operand is the reversed kernel chunk
Krev[dc][k, o] = Ktot[dc*128 + 127 - k, o] ([128, 512]); the stationary
operand is a shifted-window ("mega") view of the signal built by a single
replicating DMA: mega[k, (tau, b)] = xpad[b, tau + k] — 128 SBUF partitions
hold 128 relatively-shifted copies. The host pre-interleaves x in groups of
4 batch rows (b innermost) so both the mega DMA and every stationary slice
are contiguous. PSUM accumulates the lag-chunk chain in fp32; eviction
fuses the bias add on VectorE.
"""

import numpy as np
import ml_dtypes

BSZ, T, D_IN = 256, 512, 1
S, O, KX = 512, 512, 5
NCORES = 8
BLOC = BSZ // NCORES        # 32 batch rows per core
NBG = BLOC // 4             # 8 groups of 4 batch rows
XPW = 640                   # padded signal width: 127 zeros + 512 + 1 slack

_prog_cache = {}
LAST_RESULTS = None         # BassKernelResults of the most recent run


def _build_program(n_bg):
    import concourse.bacc as bacc
    import concourse.bass as bass
    import concourse.mybir as mybir
    from concourse.tile import TileContext

    f32 = mybir.dt.float32
    bf16 = mybir.dt.bfloat16

    nc = bacc.Bacc("TRN2", target_bir_lowering=False, debug=False)
    # xint[g, i, b] = xpad[g*4 + b, i]  (b-interleaved padded signal)
    xint = nc.dram_tensor("xint", [n_bg, XPW, 4], bf16, kind="ExternalInput")
    krev = nc.dram_tensor("krev", [4, 128, O], bf16, kind="ExternalInput")
    biasrep = nc.dram_tensor("biasrep", [16, 128, O], f32, kind="ExternalInput")
    out = nc.dram_tensor("out", [4 * n_bg, T, O], f32, kind="ExternalOutput")

    with TileContext(nc) as tc:
        with (
            tc.tile_pool(name="consts", bufs=1) as cpool,
            tc.tile_pool(name="mega", bufs=n_bg) as mpool,
            tc.tile_pool(name="osb", bufs=6) as opool,
            tc.tile_pool(name="ps", bufs=8, space="PSUM") as ppool,
        ):
            # Load order matters: the PE stream needs krev + mega[0] ASAP.
            # All input loads go on the sync (SP HWDGE) ring; bias goes on the
            # gpsimd (SWDGE) ring; output stores use the scalar (ACT HWDGE)
            # ring — three disjoint FIFO rings so stores never head-of-line
            # block the mega prefetches.
            krev_sb = cpool.tile([128, 4, O], bf16, tag="krev")
            nc.sync.dma_start(out=krev_sb[:], in_=krev.ap().rearrange("d k o -> k d o"))
            megas = []
            for bg in range(n_bg):
                # mega[k, tau, b] = xint[bg, tau + k, b]; per-partition the
                # (tau, b) free block is one contiguous 2048-elem window.
                mega = mpool.tile([128, T, 4], bf16, tag="mega")
                src = bass.AP(xint, bg * XPW * 4, [[4, 128], [4, T], [1, 4]])
                nc.sync.dma_start(out=mega[:], in_=src)
                megas.append(mega)
            bias_sb = cpool.tile([128, 16, O], f32, tag="bias")
            for i in range(4):
                nc.gpsimd.dma_start(
                    out=bias_sb[:, 4 * i : 4 * i + 4, :],
                    in_=biasrep.ap()[4 * i : 4 * i + 4].rearrange("i p o -> p i o"),
                )
            for bg in range(n_bg):
                megaf = megas[bg][:].rearrange("p t b -> p (t b)")
                for tci in range(4):
                    for mt in range(4):
                        ps = ppool.tile([128, O], f32)
                        for dc in range(tci + 1):
                            q = tci - dc
                            # lhsT[k, m=(t_rel, b)] = mega[k, q*128+mt*32+t_rel, b]
                            lhsT = megaf[:, q * 512 + mt * 128 : q * 512 + mt * 128 + 128]
                            nc.tensor.matmul(
                                ps[:],
                                lhsT,
                                krev_sb[:, dc, :],
                                start=(dc == 0),
                                stop=(dc == tci),
                            )
                        osb = opool.tile([128, O], f32)
                        nc.vector.tensor_add(
                            out=osb[:], in0=ps[:], in1=bias_sb[:, tci * 4 + mt, :]
                        )
                        # partition p = t_rel*4 + b -> out[bg*4+b, tci*128+mt*32+t_rel, :]
                        dst = bass.AP(
                            out,
                            bg * 4 * T * O + (tci * 128 + mt * 32) * O,
                            [[O, 32], [T * O, 4], [1, O]],
                        )
                        nc.scalar.dma_start(out=dst, in_=osb[:])
    nc.compile()
    return nc


def _get_program(n_bg=NBG):
    if n_bg not in _prog_cache:
        _prog_cache[n_bg] = _build_program(n_bg)
    return _prog_cache[n_bg]


def host_prep(inputs, A, B, C, M, h0):
    """float64 host precompute of the conv kernel, bias, and padded signal."""
    x = inputs[:, :, 0].astype(np.float64)          # [BSZ, T]
    A64 = A.astype(np.float64)
    B64 = B.astype(np.float64)
    C64 = C.astype(np.float64)
    M64 = M.astype(np.float64)
    h64 = h0.astype(np.float64)

    Apow = A64[None, :] ** np.arange(T + 1)[:, None]      # [T+1, S]
    K = (B64[0][None, :] * Apow[:T]) @ C64                # [T, O]
    K[1 : KX + 1, :] += M64[:, 0, :].T                    # AR taps, lags 1..KX
    bias = (h64[None, :] * Apow[1 : T + 1]) @ C64         # [T, O]

    krev = np.ascontiguousarray(
        K.reshape(4, 128, O)[:, ::-1, :]
    ).astype(ml_dtypes.bfloat16)                          # [4, 128, O]
    # biasrep[ti, t_rel*4 + b, o] = bias[ti*32 + t_rel, o], b = 0..3
    biasrep = np.ascontiguousarray(
        np.repeat(bias.reshape(16, 32, O), 4, axis=1)
    ).astype(np.float32)                                  # [16, 128, O]
    xpad = np.zeros((BSZ, XPW), np.float32)
    xpad[:, 127 : 127 + T] = x
    xpad = xpad.astype(ml_dtypes.bfloat16)                # [BSZ, XPW]
    # xint[g, i, b] = xpad[g*4 + b, i]
    xint = np.ascontiguousarray(
        xpad.reshape(BSZ // 4, 4, XPW).transpose(0, 2, 1)
    )                                                     # [BSZ//4, XPW, 4]
    return xint, krev, biasrep


def kernel(inputs, A, B, C, M, h0):
    global LAST_RESULTS
    from concourse.bass_utils import run_bass_kernel_spmd

    xint, krev, biasrep = host_prep(inputs, A, B, C, M, h0)
    nc = _get_program(NBG)
    in_maps = [
        {
            "xint": np.ascontiguousarray(xint[c * NBG : (c + 1) * NBG]),
            "krev": krev,
            "biasrep": biasrep,
        }
        for c in range(NCORES)
    ]
    res = run_bass_kernel_spmd(nc, in_maps, core_ids=list(range(NCORES)))
    LAST_RESULTS = res
    return np.concatenate([r["out"] for r in res.results], axis=0)



# revision 4
# speedup vs baseline: 1.3845x; 1.3845x over previous
"""LDS forward kernel for Trainium2 (8 NeuronCores, data-parallel over batch).

Math: the reference LDS with diagonal A and d_in == 1 is a causal conv plus
a batch-independent bias:
    out[b,t,o] = sum_{d=0}^{t} Ktot[d,o] * x[b,t-d] + bias[t,o]
    Ktot[d,o]  = sum_s B[s] A[s]^d C[s,o]  (+ M[o,0,d-1] for d in 1..KX)
    bias[t,o]  = sum_s h0[s] A[s]^{t+1} C[s,o]
Ktot is numerically tiny-rank (sum of decaying exponentials + KX delta
taps): sigma_16/sigma_0 ~ 3e-5. Host computes (f64) a rank-RK factorization
Ktot ~= U @ V (U [T, RK] orthonormal lag-modes) and a rank-RB bias
factorization bias ~= Wb @ Vb.

Device pipeline per core (32 batch rows, 8 groups of 4):
  1. conv: Z[r, t] = sum_d U[d,r] x[t-d] via blocked Toeplitz matmuls.
     Stationary = reversed mode chunk Urev[dc] [128, RK]; moving = the
     shifted-window "mega" view of the signal (one replicating DMA per
     group). 10 matmuls per group accumulate 4 PSUM tiles [RK, 512].
  2. Z eviction (DVE): PSUM -> SBUF bf16 into Zsb[tci] [32, 512] rows
     0..RK-1; rows RK..31 hold precomputed bias-mode rows Wb[t, j]
     (small DMA per tile).
  3. proj: one matmul per output tile [128, 512]: lhsT = Zsb window
     [32, 128] (contraction = RK conv modes + RB bias modes -> bias is
     folded into the matmul, no separate add), rhs = [V; Vb] [32, 512].
  4. Y eviction: PSUM fp32 -> SBUF bf16 copies alternating DVE/ACT.
  5. Output DMA: one batched store per (group, tci) from a [128, 2048]
     staging tile; out dtype bf16, upcast to fp32 on host.
"""

import numpy as np
import ml_dtypes

BSZ, T, D_IN = 256, 512, 1
S, O, KX = 512, 512, 5
NCORES = 8
BLOC = BSZ // NCORES        # 32 batch rows per core
NBG = BLOC // 4             # 8 groups of 4 batch rows
XPW = 640                   # padded signal width: 127 zeros + 512 + 1 slack
RK = 24                     # conv kernel modes
RB = 8                      # bias modes
RT = RK + RB                # proj contraction rows = 32

_prog_cache = {}
LAST_RESULTS = None         # BassKernelResults of the most recent run


def _build_program(n_bg):
    import concourse.bacc as bacc
    import concourse.bass as bass
    import concourse.mybir as mybir
    from concourse.tile import TileContext

    f32 = mybir.dt.float32
    bf16 = mybir.dt.bfloat16

    nc = bacc.Bacc("TRN2", target_bir_lowering=False, debug=False)
    # xint[g, i, b] = xpad[g*4 + b, i]  (b-interleaved padded signal)
    xint = nc.dram_tensor("xint", [n_bg, XPW, 4], bf16, kind="ExternalInput")
    urev = nc.dram_tensor("urev", [128, 4, RK], bf16, kind="ExternalInput")
    vcomb = nc.dram_tensor("vcomb", [RT, O], bf16, kind="ExternalInput")
    # wbias[tci][j, (mt*32+t_rel)*4 + b] = Wb[tci*128 + mt*32 + t_rel, j]
    wbias = nc.dram_tensor("wbias", [4, RB, 512], bf16, kind="ExternalInput")
    out = nc.dram_tensor("out", [4 * n_bg, T, O], bf16, kind="ExternalOutput")

    with TileContext(nc) as tc:
        with (
            tc.tile_pool(name="consts", bufs=1) as cpool,
            tc.tile_pool(name="mega", bufs=n_bg) as mpool,
            tc.tile_pool(name="zsb", bufs=8) as zpool,
            tc.tile_pool(name="osb", bufs=3) as opool,
            tc.tile_pool(name="zp", bufs=1, space="PSUM") as zppool,
            tc.tile_pool(name="yp", bufs=4, space="PSUM") as yppool,
        ):
            # Consts on the sync (SP HWDGE) ring; PE needs urev first.
            urev_sb = cpool.tile([128, 4, RK], bf16, tag="urev")
            nc.sync.dma_start(out=urev_sb[:], in_=urev.ap())
            vcomb_sb = cpool.tile([RT, O], bf16, tag="vcomb")
            nc.sync.dma_start(out=vcomb_sb[:], in_=vcomb.ap())
            megas = []
            for bg in range(n_bg):
                # mega[k, tau, b] = xint[bg, tau + k, b]; 128 relatively
                # shifted copies of the signal via one replicating DMA.
                mega = mpool.tile([128, T, 4], bf16, tag="mega")
                src = bass.AP(xint, bg * XPW * 4, [[4, 128], [4, T], [1, 4]])
                nc.sync.dma_start(out=mega[:], in_=src)
                megas.append(mega)

            for bg in range(n_bg):
                megaf = megas[bg][:].rearrange("p t b -> p (t b)")
                # ---- conv: Z[tci] [RK, 512] accumulated over lag chunks ----
                zps = []
                zsbs = []
                for tci in range(4):
                    zp = zppool.tile([128, 512], f32, tag=f"zp{tci}")
                    for dc in range(tci + 1):
                        q = tci - dc
                        nc.tensor.matmul(
                            zp[0:RK, :],
                            urev_sb[:, dc, :],
                            megaf[:, q * 512 : (q + 1) * 512],
                            start=(dc == 0),
                            stop=(dc == tci),
                        )
                    zps.append(zp)
                    # Zsb rows RK..RT-1 = bias-mode rows (SWDGE, tiny)
                    zsb = zpool.tile([RT, 512], bf16, tag=f"zsb{tci}")
                    nc.gpsimd.dma_start(
                        out=zsb[RK:RT, :], in_=wbias.ap()[tci]
                    )
                    zsbs.append(zsb)
                for tci in range(4):
                    # Z eviction: fp32 PSUM -> bf16 SBUF (DVE)
                    nc.vector.tensor_copy(
                        out=zsbs[tci][0:RK, :], in_=zps[tci][0:RK, :]
                    )
                # ---- proj + Y evict + batched store per tci ----
                for tci in range(4):
                    osb = opool.tile([128, 4, O], bf16, tag="osb")
                    # cols of zsb are (t_rel*4 + b); a stride-4 slice gives
                    # the 128 pure-t columns of one batch row -> Y psum
                    # partitions are consecutive t, so the store AP is 3-dim.
                    zview = zsbs[tci][:].rearrange("r (t b) -> r b t", b=4)
                    for b in range(4):
                        yp = yppool.tile([128, 512], f32)
                        nc.tensor.matmul(
                            yp[:],
                            zview[:, b, :],
                            vcomb_sb[:],
                            start=True,
                            stop=True,
                        )
                        # Y eviction alternates DVE / ACT
                        if b % 2 == 0:
                            nc.vector.tensor_copy(out=osb[:, b, :], in_=yp[:])
                        else:
                            nc.scalar.copy(out=osb[:, b, :], in_=yp[:])
                    # one batched store per (bg, tci): 512 descriptors
                    # partition p = t_rel (consecutive t); free = (b, o)
                    dst = bass.AP(
                        out,
                        bg * 4 * T * O + tci * 128 * O,
                        [[O, 128], [T * O, 4], [1, O]],
                    )
                    nc.scalar.dma_start(out=dst, in_=osb[:])
    nc.compile()
    return nc


def _get_program(n_bg=NBG):
    if n_bg not in _prog_cache:
        _prog_cache[n_bg] = _build_program(n_bg)
    return _prog_cache[n_bg]


def host_prep(inputs, A, B, C, M, h0):
    """float64 host precompute: rank factorizations + padded signal."""
    x = inputs[:, :, 0].astype(np.float64)          # [BSZ, T]
    A64 = A.astype(np.float64)
    B64 = B.astype(np.float64)
    C64 = C.astype(np.float64)
    M64 = M.astype(np.float64)
    h64 = h0.astype(np.float64)

    Apow = A64[None, :] ** np.arange(T + 1)[:, None]      # [T+1, S]
    K = (B64[0][None, :] * Apow[:T]) @ C64                # [T, O]
    K[1 : KX + 1, :] += M64[:, 0, :].T                    # AR taps, lags 1..KX
    bias = (h64[None, :] * Apow[1 : T + 1]) @ C64         # [T, O]

    UK, SK, VKt = np.linalg.svd(K, full_matrices=False)
    U = UK[:, :RK]                                        # [T, RK] orthonormal
    V = SK[:RK, None] * VKt[:RK]                          # [RK, O]
    Ub, Sb, Vbt = np.linalg.svd(bias, full_matrices=False)
    Wb = Ub[:, :RB]                                       # [T, RB]
    Vb = Sb[:RB, None] * Vbt[:RB]                         # [RB, O]

    # urev[k, dc, r] = U[dc*128 + 127 - k, r]
    urev = np.ascontiguousarray(
        U.reshape(4, 128, RK)[:, ::-1, :].transpose(1, 0, 2)
    ).astype(ml_dtypes.bfloat16)                          # [128, 4, RK]
    vcomb = np.concatenate([V, Vb], axis=0).astype(ml_dtypes.bfloat16)
    # wbias[tci, j, (mt*32+t_rel)*4 + b] = Wb[tci*128 + ..., j]
    wb = np.repeat(Wb.reshape(4, 128, RB), 4, axis=1)     # [4, 512, RB]
    wbias = np.ascontiguousarray(wb.transpose(0, 2, 1)).astype(
        ml_dtypes.bfloat16
    )                                                     # [4, RB, 512]

    xpad = np.zeros((BSZ, XPW), np.float32)
    xpad[:, 127 : 127 + T] = x
    xpad = xpad.astype(ml_dtypes.bfloat16)                # [BSZ, XPW]
    # xint[g, i, b] = xpad[g*4 + b, i]
    xint = np.ascontiguousarray(
        xpad.reshape(BSZ // 4, 4, XPW).transpose(0, 2, 1)
    )                                                     # [BSZ//4, XPW, 4]
    return xint, urev, vcomb, wbias


def kernel(inputs, A, B, C, M, h0):
    global LAST_RESULTS
    from concourse.bass_utils import run_bass_kernel_spmd

    xint, urev, vcomb, wbias = host_prep(inputs, A, B, C, M, h0)
    nc = _get_program(NBG)
    in_maps = [
        {
            "xint": np.ascontiguousarray(xint[c * NBG : (c + 1) * NBG]),
            "urev": urev,
            "vcomb": vcomb,
            "wbias": wbias,
        }
        for c in range(NCORES)
    ]
    res = run_bass_kernel_spmd(nc, in_maps, core_ids=list(range(NCORES)))
    LAST_RESULTS = res
    return np.concatenate(
        [r["out"].astype(np.float32) for r in res.results], axis=0
    )
